# revision 1
# baseline (speedup 1.0000x reference)
"""Difference-attention Trainium2 kernel (8-core SPMD, query-sharded).

Math (per batch=1):
  q_ = q @ Wq;  kv_ = kv @ Wkv;  k, v = split(kv_)
  score[i,j] = (sum_c exp(-|q_[i,c]-k[j,c]|) + sum_d wsum[d]*exp(-|qc[i,d]-kvc[j,d]|)) / C
    (uses pos_enc.sum(-1) == exp(-|dq-dk|) @ Wdelta.sum(1), so the [Nq,Nkv,C]
     pos_enc tensor is never materialized)
  out = softmax(score) @ v @ Wproj + bproj

Sharding: queries split 512 -> 8 x 64; kv/weights replicated; no collectives.

Device layout (per core):
  kT2  [128=(half h x chan c), 1408]  k transposed, kv rows [0,1408) in
       partitions 0:64, rows [1408,2816) in partitions 64:128 (bf16)
  per query i: a = |kT2 - q_i| (one fused DVE tensor_scalar, 4x bf16)
               e = exp(-a)     (ACT, grouped G queries per instr)
               PE ones2-matmul reduces channels -> psum[j-chunk, 2i+h]
  coord part: 32 queries packed per instruction on partitions (i,d)=96,
       reduced+scattered into the same psum via inline block-diag matmuls
  e2 = exp(psum/ C) -> attn@[v|1] matmuls -> out/denominator -> Wproj+bias
"""

import sys

if "/opt/trn_rl_repo" not in sys.path:
    sys.path.insert(0, "/opt/trn_rl_repo")

import numpy as np

# ---- problem constants (hardcoded per the harness contract) ----
B = 1
NQ = 512
NKV = 2562
C = 64
ICO = 64
NCORES = 8
NQL = NQ // NCORES          # 64 queries per core
NJP = 2816                  # padded kv count (22 * 128)
NH = NJP // 2               # 1408, half (partition-packed)
NT = NJP // 128             # 22 kv tiles of 128
NCH = NH // 128             # 11 column chunks per half
GACT = 7                    # queries per ACT exp-group instruction
NMINGRP = 1                 # trailing query groups computed via the
                            # DVE-only min(e^k*e^-q, e^-k*e^q) path
NCG = 32                    # queries per coord group (2 groups of 32)
SCALE = 1.0 / C

_COMPILED = None            # (nc, out_name) cache


def _build_program():
    import concourse.bass as bass
    import concourse.tile as tile
    from concourse import bacc, mybir

    F32 = mybir.dt.float32
    BF16 = mybir.dt.bfloat16
    U16 = mybir.dt.uint16
    ALU = mybir.AluOpType
    ACTF = mybir.ActivationFunctionType
    AX = mybir.AxisListType
    PSUM = bass.MemorySpace.PSUM

    nc = bacc.Bacc("TRN2", target_bir_lowering=False, debug=False,
                   num_devices=NCORES)

    # ---- external I/O ----
    q_d = nc.dram_tensor("q", [NQL, C], F32, kind="ExternalInput")
    qc_d = nc.dram_tensor("q_coord", [NQL, 3], F32, kind="ExternalInput")
    kv_d = nc.dram_tensor("kv", [NKV, ICO], F32, kind="ExternalInput")
    kvc_d = nc.dram_tensor("kv_coord", [NKV, 3], F32, kind="ExternalInput")
    wq_d = nc.dram_tensor("Wq", [C, C], F32, kind="ExternalInput")
    wkv_d = nc.dram_tensor("Wkv", [ICO, 2 * C], F32, kind="ExternalInput")
    wd_d = nc.dram_tensor("Wdelta", [3, C], F32, kind="ExternalInput")
    wp_d = nc.dram_tensor("Wproj", [C, C], F32, kind="ExternalInput")
    bp_d = nc.dram_tensor("bproj", [C, 1], F32, kind="ExternalInput")
    out_d = nc.dram_tensor("out", [NQL, C], F32, kind="ExternalOutput")

    # ---- inline compile-time constants ----
    ident_d = nc.inline_tensor(np.eye(128, dtype=np.float32), name="ident")
    # iblock2[g][h]: [96,128] maps coord partitions (ib,d) -> psum col 64g+2ib+h
    ib2_np = np.zeros((2, 2, 96, 128), dtype=np.float32)
    for g in range(2):
        for h in range(2):
            for ib in range(NCG):
                for d in range(3):
                    ib2_np[g, h, 3 * ib + d, 64 * g + 2 * ib + h] = 1.0
    import ml_dtypes as _mld
    ib2_cat = np.concatenate(
        [ib2_np[g, h] for g in range(2) for h in range(2)], axis=1)
    ib2_d = nc.inline_tensor(ib2_cat.astype(_mld.bfloat16), name="ib2cat")

    # ---- DRAM scratch for partition-replication round trips ----
    s_q = nc.dram_tensor("s_q", [C, NQL], F32)       # q_^T compact
    s_kc = nc.dram_tensor("s_kc", [3, NJP], BF16)     # kv_coord^T compact
    s_ws = nc.dram_tensor("s_ws", [3, 1], F32)        # Wdelta row sums

    with tile.TileContext(nc) as tc:
        with (
            tc.tile_pool(name="consts", bufs=1) as cp,
            tc.tile_pool(name="big", bufs=1) as bigp,
            tc.tile_pool(name="work", bufs=3) as wp,
            tc.tile_pool(name="prep", bufs=1) as pp,
            tc.tile_pool(name="abuf", bufs=2) as ap_,
            tc.tile_pool(name="ebuf", bufs=2) as ep_,
        ):
            # ---------------- kv first: it heads the critical path ---------
            kv_sb = bigp.tile([128, NT * ICO], F32, tag="kv_sb")
            nfull = NT - 2
            kv3 = kv_d.ap()[0:nfull * 128, :].rearrange("(t p) c -> p t c",
                                                        p=128)
            qeng = (nc.sync, nc.scalar)
            for s4 in range(4):
                t0, t1 = 5 * s4, min(5 * (s4 + 1), nfull)
                qeng[s4 % 2].dma_start(kv_sb[:, t0 * ICO:t1 * ICO],
                                       kv3[:, t0:t1, :])
            nc.vector.memset(kv_sb[:, nfull * ICO:NT * ICO], 0.0)
            nc.sync.dma_start(kv_sb[0:NKV - nfull * 128,
                                    nfull * ICO:nfull * ICO + ICO],
                              kv_d.ap()[nfull * 128:NKV, :])

            # ---------------- constants / weights ----------------
            ident = cp.tile([128, 128], F32, tag="ident")
            nc.scalar.dma_start(ident[:], ident_d.ap())
            wq = cp.tile([C, C], F32, tag="wq")
            nc.scalar.dma_start(wq[:], wq_d.ap())
            wkv = cp.tile([ICO, 2 * C], F32, tag="wkv")
            nc.sync.dma_start(wkv[:], wkv_d.ap())
            # high-partition copy for odd-tile matmuls (lhsT/rhs must share
            # base partition)
            wkv_hi = cp.tile([128, 2 * C], F32, tag="wkv_hi")
            nc.sync.dma_start(wkv_hi[64:128, :], wkv_d.ap())
            # Wkv_kz: cols 0:64 zero, cols 64:128 = k-cols of Wkv
            wkvkz = cp.tile([ICO, 128], F32, tag="wkvkz")
            nc.vector.memset(wkvkz[:, 0:C], 0.0)
            nc.sync.dma_start(wkvkz[:, C:2 * C], wkv_d.ap()[:, 0:C])
            wkvkz_hi = cp.tile([128, 128], F32, tag="wkvkz_hi")
            nc.vector.memset(wkvkz_hi[64:128, 0:C], 0.0)
            nc.sync.dma_start(wkvkz_hi[64:128, C:2 * C], wkv_d.ap()[:, 0:C])
            wproj = cp.tile([C, C], F32, tag="wproj")
            nc.scalar.dma_start(wproj[:], wp_d.ap())
            bproj = cp.tile([C, 1], F32, tag="bproj")
            nc.scalar.dma_start(bproj[:], bp_d.ap())
            wd = cp.tile([3, C], F32, tag="wd")
            nc.scalar.dma_start(wd[:], wd_d.ap())
            ib2c = cp.tile([96, 512], BF16, tag="ib2c")
            nc.scalar.dma_start(ib2c[:], ib2_d.ap())
            ones2 = cp.tile([128, 2], BF16, tag="ones2")
            nc.vector.memset(ones2[:], 0.0)
            nc.vector.memset(ones2[0:64, 0:1], 1.0)
            nc.vector.memset(ones2[64:128, 1:2], 1.0)

            # wsum = Wdelta.sum(axis=1) -> replicate to [96,1]
            wsum = cp.tile([3, 1], F32, tag="wsum")
            nc.vector.tensor_reduce(wsum[:], wd[:], axis=AX.X, op=ALU.add)
            nc.scalar.dma_start(s_ws.ap(), wsum[:])
            wrep = cp.tile([96, 1], F32, tag="wrep")
            nc.scalar.dma_start(
                wrep[:], s_ws.ap().unsqueeze(0).broadcast_to([NCG, 3, 1]))

            # qc groups: [96,1] scalars, partition (ib,d), group g = queries
            # [32g, 32g+32)
            qcg = [cp.tile([96, 1], F32, tag=f"qcg{g}", name=f"qcg{g}")
                   for g in (0, 1)]
            for g in (0, 1):
                nc.scalar.dma_start(
                    qcg[g][:],
                    qc_d.ap()[NCG * g:NCG * (g + 1), :].unsqueeze(2))

            # ---------------- persistent big tensors ----------------
            kT2 = bigp.tile([128, NH], BF16, tag="kT2")
            vext = bigp.tile([128, NT * 65], F32, tag="vext")
            kcrep = bigp.tile([96, NJP], BF16, tag="kcrep")
            ecw = bigp.tile([96, 2 * NJP], BF16, tag="ecw")
            qT2 = bigp.tile([128, NQL], F32, tag="qT2")
            e2 = bigp.tile([128, NH], F32, tag="e2")

            # ---------------- phase A: q path, kv path, coord prep ----------
            with tc.tile_pool(name="psA", bufs=2, space=PSUM) as psA, \
                 tc.tile_pool(name="tmpA", bufs=1) as tmpA:
                # q_ = q @ Wq, transposed and replicated across both halves
                q_sb = tmpA.tile([NQL, C], F32, tag="q_sb")
                nc.sync.dma_start(q_sb[:], q_d.ap())
                p_qt = psA.tile([C, NQL], F32, tag="pq")
                nc.tensor.transpose(p_qt[:], q_sb[:], ident[0:NQL, 0:NQL])
                qt_sb = tmpA.tile([C, NQL], F32, tag="qt_sb")
                nc.scalar.copy(qt_sb[:], p_qt[:])
                p_q2 = psA.tile([C, NQL], F32, tag="pq")
                nc.tensor.matmul(p_q2[:], wq[:], qt_sb[:], start=True, stop=True)
                qta = tmpA.tile([C, NQL], F32, tag="qta")
                nc.scalar.copy(qta[:], p_q2[:])
                nc.sync.dma_start(s_q.ap(), qta[:])
                nc.scalar.dma_start(
                    qT2[:], s_q.ap().unsqueeze(0).broadcast_to([2, C, NQL]))

                # kv_coord^T gather + pad + replicate to [96, NJP]
                kvcT = tmpA.tile([3, NJP], F32, tag="kvcT")
                nc.sync.dma_start(kvcT[:, 0:NKV], kvc_d.ap().transpose([1, 0]))
                nc.vector.memset(kvcT[:, NKV:NJP], 0.0)
                kvcT_bf = tmpA.tile([3, NJP], BF16, tag="kvcTbf")
                nc.vector.tensor_copy(kvcT_bf[:], kvcT[:])
                nc.sync.dma_start(s_kc.ap(), kvcT_bf[:])
                nc.sync.dma_start(
                    kcrep[0:48, :],
                    s_kc.ap().unsqueeze(0).broadcast_to([16, 3, NJP]))
                nc.scalar.dma_start(
                    kcrep[48:96, :],
                    s_kc.ap().unsqueeze(0).broadcast_to([16, 3, NJP]))

                # coord elementwise: acg = |kcrep - qc|, ecw = wsum*exp(-acg)
                for g in (0, 1):
                    dcg = tmpA.tile([96, NJP], BF16, tag="dcg", bufs=2)
                    nc.vector.tensor_scalar(dcg[:], kcrep[:], qcg[g][:],
                                            None, ALU.subtract)
                    acg = tmpA.tile([96, NJP], BF16, tag="acg", bufs=2)
                    nc.vector.tensor_scalar(acg[:].bitcast(U16),
                                            dcg[:].bitcast(U16),
                                            0x7FFF, None, ALU.bitwise_and)
                    esl = ecw[:, g * NJP:(g + 1) * NJP]
                    nc.scalar.activation(esl, acg[:], ACTF.Exp, scale=-1.0)
                    nc.vector.tensor_scalar(esl, esl, wrep[:], None, ALU.mult)

                # kv tiles: pair-batched transposes (2 tiles per PE op);
                # kvT_all[64s:64s+64, pr*128:+128] = (tile 2*pr+s)^T
                kvT_all = bigp.tile([128, (NT // 2) * 128], F32, tag="kvT_all")
                for pr in range(NT // 2):
                    p_t1 = psA.tile([128, 128], F32, tag="pt1")
                    nc.tensor.transpose(p_t1[:],
                                        kv_sb[:, pr * 128:(pr + 1) * 128],
                                        ident[:])
                    nc.vector.tensor_copy(
                        kvT_all[:, pr * 128:(pr + 1) * 128], p_t1[:])
                for t in range(NT):
                    pr, sx = t // 2, t % 2
                    kvT = kvT_all[64 * sx:64 * sx + 64,
                                  pr * 128:(pr + 1) * 128]
                    r0 = t * 128
                    # kv_ tile [128 j, 128 c2]
                    wkv_t = wkv[:] if sx == 0 else wkv_hi[64:128, :]
                    p_kv = psA.tile([128, 2 * C], F32, tag="pkv")
                    nc.tensor.matmul(p_kv[:], kvT, wkv_t, start=True,
                                     stop=True)
                    # v columns + ones column
                    vbase = t * 65
                    nc.scalar.copy(vext[:, vbase:vbase + C],
                                   p_kv[:, C:2 * C])
                    if t < NT - 2:
                        nc.gpsimd.memset(vext[:, vbase + C:vbase + 65], 1.0)
                    elif t == NT - 2:
                        nc.gpsimd.memset(vext[:, vbase + C:vbase + 65], 0.0)
                        nc.gpsimd.memset(vext[0:NKV - r0, vbase + C:vbase + 65],
                                         1.0)
                    else:
                        nc.gpsimd.memset(vext[:, vbase + C:vbase + 65], 0.0)
                    # k^T: t<11 -> partitions 0:64 of chunk t;
                    #      t>=11 -> partitions 64:128 of chunk t-11
                    p_kt = psA.tile([128, 128], F32, tag="pkt")
                    if t < NCH:
                        wkv_k = (wkv[:, 0:C] if sx == 0
                                 else wkv_hi[64:128, 0:C])
                        nc.tensor.matmul(p_kt[0:C, :], wkv_k, kvT,
                                         start=True, stop=True)
                        nc.vector.tensor_copy(kT2[0:64, t * 128:(t + 1) * 128],
                                              p_kt[0:C, :])
                    else:
                        wkvkz_t = (wkvkz[:] if sx == 0
                                   else wkvkz_hi[64:128, :])
                        nc.tensor.matmul(p_kt[:], wkvkz_t, kvT,
                                         start=True, stop=True)
                        c0 = (t - NCH) * 128
                        nc.vector.tensor_copy(kT2[64:128, c0:c0 + 128],
                                              p_kt[64:128, :])

            # ---------------- phase B: scores ----------------
            ek2 = bigp.tile([128, NH], BF16, tag="ek2")
            emk2 = bigp.tile([128, NH], BF16, tag="emk2")
            nc.scalar.activation(ek2[:], kT2[:], ACTF.Exp, scale=1.0)
            nc.scalar.activation(emk2[:], kT2[:], ACTF.Exp, scale=-1.0)
            eq = bigp.tile([128, NQL], F32, tag="eq")
            emq = bigp.tile([128, NQL], F32, tag="emq")
            nc.scalar.activation(eq[:], qT2[:], ACTF.Exp, scale=1.0)
            nc.scalar.activation(emq[:], qT2[:], ACTF.Exp, scale=-1.0)
            with tc.tile_pool(name="psM", bufs=1, space=PSUM) as psM:
                psum_main = psM.tile([128, NH], F32, tag="pmain")
                nc.vector.memset(psum_main[:], 0.0)

                # coord reductions scatter-accumulate into psum_main
                # (all start=False onto the memset zeros: a start=True
                #  matmul resets the whole PSUM bank, wiping neighbors)
                for jc in range(NCH):
                    for g in (0, 1):
                        for h in (0, 1):
                            lhs = ecw[:, g * NJP + (jc + NCH * h) * 128:
                                      g * NJP + (jc + NCH * h) * 128 + 128]
                            nc.tensor.matmul(
                                psum_main[:, jc * 128:(jc + 1) * 128],
                                lhs, ib2c[:, (2 * g + h) * 128:
                                           (2 * g + h + 1) * 128],
                                start=False, stop=False,
                                skip_group_check=True,
                            )

                # main channel-difference loop: 7 exp-path groups of 8
                # queries; the last 8 queries use the DVE-only min path,
                # interleaved between groups to fill ACT-limited DVE gaps
                nexp = 8              # 8 exp groups of GACT=7 -> 56 queries
                nminq = NQL - nexp * GACT   # 8 min-path queries

                def emit_min_query(u):
                    i = nexp * GACT + u
                    tt1 = wp.tile([128, NH], BF16, tag="tt1",
                                  name=f"tt1_{u}")
                    nc.vector.tensor_scalar(
                        tt1[:], ek2[:], emq[:, i:i + 1], None, ALU.mult)
                    tt2 = wp.tile([128, NH], BF16, tag="tt2",
                                  name=f"tt2_{u}")
                    nc.vector.tensor_scalar(
                        tt2[:], emk2[:], eq[:, i:i + 1], None, ALU.mult)
                    emn = wp.tile([128, NH], BF16, tag="emn",
                                  name=f"emn_{u}")
                    nc.vector.tensor_tensor(emn[:], tt1[:], tt2[:], ALU.min)
                    for jc in range(NCH):
                        nc.tensor.matmul(
                            psum_main[:, jc * 128 + 2 * i:
                                      jc * 128 + 2 * i + 2],
                            emn[:, jc * 128:(jc + 1) * 128],
                            ones2[:], start=False, stop=True,
                            skip_group_check=True)

                for gq in range(nexp):
                    ebuf = ep_.tile([128, GACT * NH], BF16, tag="ebuf")
                    abuf = ap_.tile([128, GACT * NH], BF16, tag="abuf")
                    for u in range(GACT):
                        i = gq * GACT + u
                        dd = wp.tile([128, NH], BF16, tag="dd")
                        nc.vector.tensor_scalar(
                            dd[:], kT2[:], qT2[:, i:i + 1], None,
                            ALU.subtract)
                        nc.vector.tensor_scalar(
                            abuf[:, u * NH:(u + 1) * NH].bitcast(U16),
                            dd[:].bitcast(U16), 0x7FFF, None,
                            ALU.bitwise_and)
                    nc.scalar.activation(ebuf[:], abuf[:], ACTF.Exp,
                                         scale=-1.0)
                    if gq >= nexp - 4:
                        emit_min_query(gq - (nexp - 4))
                    for u in range(GACT):
                        i = gq * GACT + u
                        for jc in range(NCH):
                            nc.tensor.matmul(
                                psum_main[:, jc * 128 + 2 * i:
                                          jc * 128 + 2 * i + 2],
                                ebuf[:, u * NH + jc * 128:
                                     u * NH + (jc + 1) * 128],
                                ones2[:],
                                start=False, stop=True,
                                skip_group_check=True,
                            )

                for u in range(4, nminq):
                    emit_min_query(u)

                # e2 = exp(score/C) over the whole interleaved psum
                nc.scalar.activation(e2[:], psum_main[:], ACTF.Exp,
                                     scale=SCALE)

            # ---------------- phase C: attn @ [v|1], divide, proj ----------
            with tc.tile_pool(name="psC", bufs=1, space=PSUM) as psC:
                p_att = psC.tile([NQL, 65], F32, tag="patt")
                for jc in range(NCH):
                    for h in (0, 1):
                        t = jc + NCH * h
                        lhs = e2[:, jc * 128 + h: (jc + 1) * 128: 2]
                        nc.tensor.matmul(
                            p_att[:], lhs, vext[:, t * 65:(t + 1) * 65],
                            start=(jc == 0 and h == 0),
                            stop=(jc == NCH - 1 and h == 1),
                        )
                rs = wp.tile([NQL, 1], F32, tag="rs")
                nc.vector.reciprocal(rs[:], p_att[:, C:C + 1])
                o_sb = wp.tile([NQL, C], F32, tag="o_sb")
                nc.vector.tensor_scalar(o_sb[:], p_att[:, 0:C], rs[:], None,
                                        ALU.mult)
                # proj: out = o @ Wproj + bproj
                p_ot = psC.tile([C, NQL], F32, tag="pot")
                nc.tensor.transpose(p_ot[:], o_sb[:], ident[0:NQL, 0:NQL])
                ot_sb = wp.tile([C, NQL], F32, tag="ot_sb")
                nc.vector.tensor_copy(ot_sb[:], p_ot[:])
                p_pj = psC.tile([C, NQL], F32, tag="ppj")
                nc.tensor.matmul(p_pj[:], wproj[:], ot_sb[:], start=True,
                                 stop=True)
                pj_sb = wp.tile([C, NQL], F32, tag="pj_sb")
                nc.vector.tensor_scalar(pj_sb[:], p_pj[:], bproj[:], None,
                                        ALU.add)
                p_o2 = psC.tile([NQL, C], F32, tag="po2")
                nc.tensor.transpose(p_o2[:], pj_sb[:], ident[0:C, 0:C])
                po_sb = wp.tile([NQL, C], F32, tag="po_sb")
                nc.vector.tensor_copy(po_sb[:], p_o2[:])
                nc.sync.dma_start(out_d.ap(), po_sb[:])

    nc.compile()
    return nc


def _get_compiled():
    global _COMPILED
    if _COMPILED is None:
        _COMPILED = _build_program()
    return _COMPILED


def _make_in_maps(inputs):
    q = np.asarray(inputs["q"], np.float32).reshape(NQ, C)
    qc = np.asarray(inputs["q_coord"], np.float32).reshape(NQ, 3)
    kv = np.asarray(inputs["kv"], np.float32).reshape(NKV, ICO)
    kvc = np.asarray(inputs["kv_coord"], np.float32).reshape(NKV, 3)
    shared = {
        "kv": np.ascontiguousarray(kv),
        "kv_coord": np.ascontiguousarray(kvc),
        "Wq": np.ascontiguousarray(np.asarray(inputs["Wq"], np.float32)),
        "Wkv": np.ascontiguousarray(np.asarray(inputs["Wkv"], np.float32)),
        "Wdelta": np.ascontiguousarray(np.asarray(inputs["Wdelta"], np.float32)),
        "Wproj": np.ascontiguousarray(np.asarray(inputs["Wproj"], np.float32)),
        "bproj": np.ascontiguousarray(
            np.asarray(inputs["bproj"], np.float32).reshape(C, 1)),
    }
    in_maps = []
    for core in range(NCORES):
        sl = slice(core * NQL, (core + 1) * NQL)
        m = dict(shared)
        m["q"] = np.ascontiguousarray(q[sl])
        m["q_coord"] = np.ascontiguousarray(qc[sl])
        in_maps.append(m)
    return in_maps


def run_on_hw(inputs, trace=False, **kw):
    """Run on the 8 NeuronCores; returns (output [B,NQ,C], BassKernelResults)."""
    from concourse.bass_utils import run_bass_kernel_spmd

    nc = _get_compiled()
    in_maps = _make_in_maps(inputs)
    res = run_bass_kernel_spmd(nc, in_maps, list(range(NCORES)), trace=trace,
                               **kw)
    out = np.concatenate([r["out"] for r in res.results], axis=0)
    return out.reshape(B, NQ, C).astype(np.float32), res


def kernel(**inputs) -> np.ndarray:
    out, _ = run_on_hw(inputs, trace=False)
    return out


# ---------------------------------------------------------------------------
# Timing support: cached jitted executable with K chained NEFF executions per
# dispatch, so per-iteration device time is resolvable from wall deltas.
_TIMED = {}


def _make_chained(nreps):
    import jax
    import numpy as _np
    from jax.sharding import Mesh, PartitionSpec
    from jax.experimental.shard_map import shard_map
    from concourse import mybir
    from concourse.bass2jax import (_bass_exec_p, install_neuronx_cc_hook,
                                    partition_id_tensor)

    install_neuronx_cc_hook()
    nc = _get_compiled()
    pname = nc.partition_id_tensor.name if nc.partition_id_tensor else None
    in_names, out_names, out_avals = [], [], []
    for alloc in nc.m.functions[0].allocations:
        import concourse.mybir as mb
        if not isinstance(alloc, mb.MemoryLocationSet):
            continue
        name = alloc.memorylocations[0].name
        if alloc.kind == "ExternalInput":
            if name != pname:
                in_names.append(name)
        elif alloc.kind == "ExternalOutput":
            out_names.append(name)
            out_avals.append(jax.core.ShapedArray(
                tuple(alloc.tensor_shape), mybir.dt.np(alloc.dtype)))
    n_params = len(in_names)
    all_in_names = tuple(in_names + out_names
                         + ([pname] if pname else []))

    def _body(*args):
        ops = list(args)
        pid = [partition_id_tensor()] if pname else []
        res = None
        for _ in range(nreps):
            outs = _bass_exec_p.bind(
                *(ops + pid),
                out_avals=tuple(out_avals),
                in_names=all_in_names,
                out_names=tuple(out_names),
                lowering_input_output_aliases=(),
                sim_require_finite=True,
                sim_require_nnan=True,
                nc=nc,
            )
            res = outs
            # force a data dependency so chained executions are not DCE'd
            ops = [ops[0] + 0.0 * res[0].astype(ops[0].dtype)] + ops[1:]
        return tuple(res)

    devices = jax.devices()[:NCORES]
    mesh = Mesh(_np.asarray(devices), ("core",))
    nin = n_params + len(out_names)
    sharded = jax.jit(shard_map(
        _body, mesh=mesh, in_specs=(PartitionSpec("core"),) * nin,
        out_specs=(PartitionSpec("core"),) * len(out_names),
        check_rep=False), keep_unused=True)
    return sharded, in_names, out_names, out_avals


def timed_run(inputs, nreps=16, rounds=8):
    """Estimate per-NEFF-execution time from chained-run wall deltas."""
    import time
    import jax
    import numpy as _np

    in_maps = _make_in_maps(inputs)

    def prep(nr):
        fn, in_names, out_names, out_avals = _make_chained(nr)
        concat = [_np.concatenate([m[k] for m in in_maps], axis=0)
                  for k in in_names]
        concat += [_np.zeros((NCORES * a.shape[0], *a.shape[1:]), a.dtype)
                   for a in out_avals]
        dev = [jax.device_put(c) for c in concat]
        return fn, dev, out_names

    results = {}
    for nr in (1, nreps):
        fn, dev, out_names = prep(nr)
        outs = fn(*dev)  # compile+warm
        jax.block_until_ready(outs)
        ts = []
        for _ in range(rounds):
            t0 = time.time()
            outs = fn(*dev)
            jax.block_until_ready(outs)
            ts.append(time.time() - t0)
        results[nr] = min(ts)
    per_iter = (results[nreps] - results[1]) / (nreps - 1)
    return per_iter, results

